# revision 9
# baseline (speedup 1.0000x reference)
"""Trainium2 Bass kernel for nn_ClassAwareLoss (class-aware frame loss).

Contract: kernel(**inputs) takes the FULL unsharded inputs (numpy arrays,
keyed as in setup_inputs()) and returns the FULL output (a float32 scalar).

Strategy (data-parallel over batch, per the sharding hint):
  - Shard `input`/`target` row-wise across 8 NeuronCores (2048 samples each).
  - Replicate the small tensors (frames^T, per-frame class ids, per-frame
    cosine weights) to every core.
  - Each core computes partial sums of
        caloss_c = sum_b sum_f [class(f)==t_b] * cosine_c[t_b] * (1 - d_bf)^2
        reg_c    = sum_b (||x_b|| - 1)^2
    and the host combines: (sum caloss + 6e-4 * sum reg) / B.

Device algorithm (per core, 2048 samples):
  dots are computed in bf16 on the PE (fp32 accumulate in PSUM); the
  normalization 1/||x|| is folded into the ScalarE pass that computes
  S = (1 - g*r)^2 via activation(Square, scale=-g, bias=1).  The
  class mask and per-frame cosine weight fuse into one DVE
  scalar_tensor_tensor op: w = (frame_class == t) * cosine_c[frame_class],
  and a tensor_tensor_reduce accumulates sum(w * S) per partition.
"""

import sys
import types
from contextlib import ExitStack

sys.path.insert(0, "/opt/trn_rl_repo")

import numpy as np
import ml_dtypes

# ---------------------------------------------------------------------------
# antenv.axon_hooks shim: lets run_bass_kernel_spmd(trace=True) capture NTFF
# profiles under axon.  Harmless when BASS_TRACE is not set.
# ---------------------------------------------------------------------------
try:
    import antenv

    if "antenv.axon_hooks" not in sys.modules:
        _mod = types.ModuleType("antenv.axon_hooks")
        _hook = [None]
        _mod.set_axon_ntff_profile_hook = lambda h: _hook.__setitem__(0, h)
        _mod.get_axon_ntff_profile_hook = lambda: _hook[0]
        sys.modules["antenv.axon_hooks"] = _mod
        antenv.axon_hooks = _mod
        try:
            from trn_agent_boot.trn_boot import _ntff_profile_via_ctypes

            _mod.set_axon_ntff_profile_hook(
                _ntff_profile_via_ctypes("/opt/axon/libaxon_pjrt.so")
            )
        except Exception:
            pass
except Exception:
    pass

import concourse.bass as bass
import concourse.tile as tile
import concourse.bass_utils as bass_utils
from concourse import bacc, mybir

# No cloud bucket in this container; keep artifacts local.
bass_utils.upload_artifacts = lambda tmpdir: "local://" + tmpdir

# ---------------------------------------------------------------------------
# Problem constants (from the reference problem definition; input-independent)
# ---------------------------------------------------------------------------
N_CORES = 8
B = 16384
D = 256
NCLS = 100
F_PARAM = 17
BS = B // N_CORES            # 2048 samples per core
NT = BS // 128               # 16 sample-tiles of 128 per core
F_TOTAL = NCLS * (F_PARAM - 1)  # 1600 frame rows

_CLS_SAMPLES = [5000 - 50 * i for i in range(100)]


def _calc_cls_idx(cls_samples, f):
    nc_ = len(cls_samples)
    n_samples = sum(cls_samples)
    ca_frame_num = [int((f - 2) * nc_ * r / n_samples) + 1 for r in cls_samples]
    over_flow = nc_ * (f - 1) - sum(ca_frame_num)
    for i in range(over_flow):
        ca_frame_num[i] += 1
    ca_frame_num.reverse()
    cls_frame_idx = [sum(ca_frame_num[0:k]) for k in range(nc_ + 1)]
    return cls_frame_idx, ca_frame_num


CLS_FRAME_IDX, CA_FRAME_NUM = _calc_cls_idx(_CLS_SAMPLES, F_PARAM)
FRAME_CLASS = np.repeat(np.arange(NCLS), CA_FRAME_NUM)  # [1600], deterministic

BF16 = mybir.dt.bfloat16
F32 = mybir.dt.float32
AF = mybir.ActivationFunctionType
ALU = mybir.AluOpType

_COMPILED = None   # (nc, meta)
LAST_RESULT = None  # BassKernelResults of the most recent run (for test.py)


def _build_program():
    """Build + compile the SPMD Bass program (one program, run on 8 cores)."""
    nc = bacc.Bacc(
        "TRN2", target_bir_lowering=False, debug=False, num_devices=N_CORES
    )

    # Per-core inputs
    x_bf = nc.dram_tensor("x_bf", [BS, D], BF16, kind="ExternalInput").ap()
    t_f32 = nc.dram_tensor("t_f32", [128, NT], F32, kind="ExternalInput").ap()
    framesT = nc.dram_tensor("framesT", [D, F_TOTAL], BF16, kind="ExternalInput").ap()
    fc_mat = nc.dram_tensor("fc_mat", [128, F_TOTAL], BF16, kind="ExternalInput").ap()
    cfc_mat = nc.dram_tensor("cfc_mat", [128, F_TOTAL], BF16, kind="ExternalInput").ap()
    out = nc.dram_tensor("out", [128, 2], F32, kind="ExternalOutput").ap()

    with tile.TileContext(nc) as tc:
        with ExitStack() as ctx:
            const_pool = ctx.enter_context(tc.tile_pool(name="const", bufs=1))
            work_pool = ctx.enter_context(tc.tile_pool(name="work", bufs=1))
            s_pool = ctx.enter_context(tc.tile_pool(name="s", bufs=3))
            w_pool = ctx.enter_context(tc.tile_pool(name="w", bufs=3))
            psum_pool = ctx.enter_context(
                tc.tile_pool(name="psum", bufs=2, space="PSUM")
            )

            # ---- load replicated tensors ----
            framesT_sb = const_pool.tile([128, 2 * F_TOTAL], BF16, tag="framesT")
            nc.sync.dma_start(framesT_sb[:, 0:F_TOTAL], framesT[0:128, :])
            nc.sync.dma_start(framesT_sb[:, F_TOTAL : 2 * F_TOTAL], framesT[128:256, :])
            fc_sb = const_pool.tile([128, F_TOTAL], BF16, tag="fc")
            nc.sync.dma_start(fc_sb[:], fc_mat[:])
            cfc_sb = const_pool.tile([128, F_TOTAL], BF16, tag="cfc")
            nc.sync.dma_start(cfc_sb[:], cfc_mat[:])
            t_sb = const_pool.tile([128, NT], F32, tag="t")
            nc.sync.dma_start(t_sb[:], t_f32[:])

            neg_one = const_pool.tile([128, 1], F32, tag="negone")
            nc.vector.memset(neg_one[:], -1.0)

            # ---- x natural layout [128, NT*D] (tile i at cols i*D..) ----
            xn = work_pool.tile([128, NT * D], BF16, tag="xn")
            nc.sync.dma_start(
                xn[:].rearrange("p (i d) -> p i d", i=NT),
                x_bf.rearrange("(i p) d -> p i d", p=128),
            )

            # ---- x transposed [2 x 128, BS] via DMA xbar transpose ----
            xt0 = work_pool.tile([128, BS], BF16, tag="xt0")
            xt1 = work_pool.tile([128, BS], BF16, tag="xt1")
            nc.sync.dma_start_transpose(xt0[:], x_bf[:, 0:128])
            nc.sync.dma_start_transpose(xt1[:], x_bf[:, 128:256])

            # ---- per-sample squared norms -> [128, NT] ----
            sq = work_pool.tile([128, NT], F32, tag="sq")
            sq_dump = work_pool.tile([128, D], F32, tag="sqd")
            for i in range(NT):
                nc.scalar.activation(
                    sq_dump[:],
                    xn[:, i * D : (i + 1) * D],
                    AF.Square,
                    accum_out=sq[:, i : i + 1],
                )
            # norm, 1/norm, (norm-1)^2
            norm = work_pool.tile([128, NT], F32, tag="norm")
            nc.scalar.activation(norm[:], sq[:], AF.Sqrt)
            g = work_pool.tile([128, NT], F32, tag="g")
            nc.vector.reciprocal(g[:], norm[:])
            regsq = work_pool.tile([128, NT], F32, tag="regsq")
            nc.scalar.activation(
                regsq[:], norm[:], AF.Square, bias=neg_one[:], scale=1.0
            )
            reg_col = work_pool.tile([128, 1], F32, tag="regcol")
            nc.vector.tensor_reduce(
                out=reg_col[:], in_=regsq[:], axis=mybir.AxisListType.X, op=ALU.add
            )

            # ---- main loop over sample tiles ----
            cal_cols = work_pool.tile([128, NT], F32, tag="calcols")
            NCHUNK = (F_TOTAL + 511) // 512
            for i in range(NT):
                dots = psum_pool.tile([128, F_TOTAL], F32, tag="dots")
                for c in range(NCHUNK):
                    lo = c * 512
                    hi = min(lo + 512, F_TOTAL)
                    nc.tensor.matmul(
                        dots[:, lo:hi],
                        lhsT=xt0[:, i * 128 : (i + 1) * 128],
                        rhs=framesT_sb[:, lo:hi],
                        start=True,
                        stop=False,
                    )
                    nc.tensor.matmul(
                        dots[:, lo:hi],
                        lhsT=xt1[:, i * 128 : (i + 1) * 128],
                        rhs=framesT_sb[:, F_TOTAL + lo : F_TOTAL + hi],
                        start=False,
                        stop=True,
                    )
                # S = (g*r - 1)^2 == (1 - g*r)^2  (ScalarE: PSUM -> SBUF bf16)
                s_tile = s_pool.tile([128, F_TOTAL], BF16, tag="s")
                nc.scalar.activation(
                    s_tile[:],
                    dots[:],
                    AF.Square,
                    bias=neg_one[:],
                    scale=g[:, i : i + 1],
                )
                # w = (fc == t) * cfc      (one DVE op)
                w_tile = w_pool.tile([128, F_TOTAL], BF16, tag="w")
                nc.vector.scalar_tensor_tensor(
                    out=w_tile[:],
                    in0=fc_sb[:],
                    scalar=t_sb[:, i : i + 1],
                    in1=cfc_sb[:],
                    op0=ALU.is_equal,
                    op1=ALU.mult,
                )
                # cal_cols[:, i] = sum_f w * S
                ws_dump = w_pool.tile([128, F_TOTAL], BF16, tag="wsdump")
                nc.vector.scalar_tensor_tensor(
                    out=ws_dump[:],
                    in0=w_tile[:],
                    scalar=1.0,
                    in1=s_tile[:],
                    op0=ALU.mult,
                    op1=ALU.mult,
                    accum_out=cal_cols[:, i : i + 1],
                )

            cal_col = work_pool.tile([128, 1], F32, tag="calcol")
            nc.vector.tensor_reduce(
                out=cal_col[:], in_=cal_cols[:], axis=mybir.AxisListType.X, op=ALU.add
            )
            res_sb = work_pool.tile([128, 2], F32, tag="res")
            nc.vector.tensor_copy(res_sb[:, 0:1], cal_col[:])
            nc.vector.tensor_copy(res_sb[:, 1:2], reg_col[:])
            nc.sync.dma_start(out[:], res_sb[:])

    nc.compile()
    return nc


def _prepare_inputs(inputs):
    x = np.asarray(inputs["input"], dtype=np.float32)        # [B, D]
    frames = np.asarray(inputs["frames"], dtype=np.float32)  # [F, D]
    cosine_c = np.asarray(inputs["cosine_c"], dtype=np.float32)  # [NCLS]
    target = np.asarray(inputs["target"])                    # [B] int

    x_bf = x.astype(ml_dtypes.bfloat16)
    framesT = np.ascontiguousarray(frames.T).astype(ml_dtypes.bfloat16)  # [D, F]
    fc_row = FRAME_CLASS.astype(np.float32)                 # [F] known pattern
    cfc_row = cosine_c[FRAME_CLASS].astype(np.float32)      # [F]
    fc_mat = np.ascontiguousarray(
        np.broadcast_to(fc_row.astype(ml_dtypes.bfloat16), (128, F_TOTAL))
    )
    cfc_mat = np.ascontiguousarray(
        np.broadcast_to(cfc_row.astype(ml_dtypes.bfloat16), (128, F_TOTAL))
    )

    in_maps = []
    for c in range(N_CORES):
        sl = slice(c * BS, (c + 1) * BS)
        tc_ = target[sl].astype(np.float32).reshape(NT, 128).T  # [128, NT]
        # negate target? no: t values compared with fc via is_equal.
        in_maps.append(
            {
                "x_bf": np.ascontiguousarray(x_bf[sl]),
                "t_f32": np.ascontiguousarray(tc_),
                "framesT": framesT,
                "fc_mat": fc_mat,
                "cfc_mat": cfc_mat,
            }
        )
    return in_maps


def kernel(**inputs):
    global _COMPILED, LAST_RESULT
    if _COMPILED is None:
        _COMPILED = _build_program()
    nc = _COMPILED

    in_maps = _prepare_inputs(inputs)
    res = bass_utils.run_bass_kernel_spmd(
        nc, in_maps, core_ids=list(range(N_CORES))
    )
    LAST_RESULT = res

    caloss = 0.0
    reg = 0.0
    for c in range(N_CORES):
        o = res.results[c]["out"].astype(np.float64)
        caloss += o[:, 0].sum()
        reg += o[:, 1].sum()
    val = (caloss + 0.0006 * reg) / B
    return np.float32(val)
